# revision 24
# baseline (speedup 1.0000x reference)
"""DeepSeekMoE (E=8, top-2) on 8 TRN2 NeuronCores, expert-parallel.

Strategy (per sharding hint): host computes the gate (routing IS the
data-dependent sharding step), dispatches each token's top-2 experts to the
owning cores, pads per-expert token lists to a common capacity C. Core i runs
expert i's FFN over its gathered tokens plus the shared-expert FFN over a
512-token data-parallel slice. Host scatter-adds the weighted expert outputs
and shared outputs back to the full [B, S, D] tensor.

Device layout: activations live transposed ([feature, token]) end to end so
both matmuls use natural-layout weights as the stationary operand and no
on-device transposes are needed. All matmuls run in bf16 with f32 PSUM
accumulation; gelu+bias fuses on ScalarE at PSUM eviction; the second
matmul's eviction fuses (y + b2) * combine_weight on VectorE. Weights are
streamed just-in-time in small chunks (w1 by 512-wide h-column groups, w2 by
128-wide output-d groups) so the PE never waits long on weight DMA.
"""

import numpy as np
import ml_dtypes

import concourse.mybir as mybir
import concourse.tile as tile
from concourse import bacc
from concourse.bass_utils import run_bass_kernel_spmd

D = 1024
E = 8
TOPK = 2
H = 4096
NCORES = 8
P = 128
NCHUNK = 512  # moving-operand / PSUM-bank token chunk

BF16 = mybir.dt.bfloat16
F32 = mybir.dt.float32

_cache: dict = {}


def _chunks(c):
    out = []
    o = 0
    while o < c:
        n = min(NCHUNK, c - o)
        out.append((o, n))
        o += n
    return out


def build(C: int, S: int):
    """Build + compile the SPMD per-core program.

    C: expert token capacity (any multiple of 16). S: shared-expert tokens
    per core. Same program on all 8 cores; per-core data differs.
    """
    nc = bacc.Bacc(None, target_bir_lowering=False, debug=False)

    ND = D // P      # 8 d-chunks
    NH = H // P      # 32 h-chunks
    NHQ = NH // 4    # 8 h-quad groups (512 cols of w1 per group)

    xg = nc.declare_dram_parameter("xg", [D, C], BF16, isOutput=False)
    sx = nc.declare_dram_parameter("sx", [D, S], BF16, isOutput=False)
    # w1 chunked [hq, d, 128, 512]; w2 chunked [dt, j, 128, 4, 128]
    w1 = nc.declare_dram_parameter("w1", [NHQ, ND, P, 512], BF16, isOutput=False)
    w2 = nc.declare_dram_parameter("w2", [ND, NHQ, P, 4, P], BF16, isOutput=False)
    sw1 = nc.declare_dram_parameter("sw1", [NHQ, ND, P, 512], BF16, isOutput=False)
    sw2 = nc.declare_dram_parameter("sw2", [ND, NHQ, P, 4, P], BF16, isOutput=False)
    b1c = nc.declare_dram_parameter("b1c", [P, NH], F32, isOutput=False)
    b2c = nc.declare_dram_parameter("b2c", [P, ND], F32, isOutput=False)
    sb1c = nc.declare_dram_parameter("sb1c", [P, NH], F32, isOutput=False)
    sb2c = nc.declare_dram_parameter("sb2c", [P, ND], F32, isOutput=False)
    scale = nc.declare_dram_parameter("scale", [P, C], F32, isOutput=False)
    ye = nc.declare_dram_parameter("ye", [D, C], F32, isOutput=True)
    ys = nc.declare_dram_parameter("ys", [D, S], F32, isOutput=True)

    ech = _chunks(C)
    sch = _chunks(S)

    with tile.TileContext(nc) as tc:
        with (
            tc.tile_pool(name="wp1", bufs=24) as wp1,
            tc.tile_pool(name="wp2", bufs=24) as wp2,
            tc.tile_pool(name="xp", bufs=24) as xp,
            tc.tile_pool(name="sp", bufs=8) as sp,
            tc.tile_pool(name="hp", bufs=32) as hp,
            tc.tile_pool(name="cp", bufs=1) as cp,
            tc.tile_pool(name="op", bufs=4) as op,
            tc.tile_pool(name="pp", bufs=8, space="PSUM") as pp,
        ):
            # PE warm-up sized to end at data-ready (~9us): cold mms run
            # ~426ns, so 14 fill the DMA spin-up window and flip the HAM to
            # full clock before the first real matmul.
            wut = cp.tile([P, NCHUNK], BF16, tag="wu")
            nc.vector.memset(wut[:], 0.0)
            for wi in range(14):
                wps = pp.tile([P, NCHUNK], F32, tag="ps", name=f"wu{wi}")
                nc.tensor.matmul(wps[:], wut[:, :P], wut[:], start=True,
                                 stop=True)

            # Load order = need order, all on the sync queue so descriptor
            # order delays non-critical bytes: xg chunk 0 + first w1 chunk
            # gate the first matmul (DMAs interleave inside ffn); everything
            # else is issued just before first use.
            xgt = [[xp.tile([P, NCHUNK], BF16, tag="xg", name=f"xg{d}_{ti}")
                    for ti in range(len(ech))] for d in range(ND)]
            b1t = cp.tile([P, NH], F32, tag="b1")
            sb1t = cp.tile([P, NH], F32, tag="sb1")
            b2t = cp.tile([P, ND], F32, tag="b2")
            sb2t = cp.tile([P, ND], F32, tag="sb2")
            sct = cp.tile([P, C], F32, tag="scale")
            sxt = [[sp.tile([P, S], BF16, tag="sx", name=f"sx{d}")]
                   for d in range(ND)]

            def ffn(tagp, w1_ap, w2_ap, b1_tile, b2_tile, x_tiles, chs, y_ap,
                    sc_tile, pre2=None, x_dma=None):
                # phase 1: hT[h, tok] = gelu(w1[:,h].T @ x + b1[h])
                hts = []
                for hq in range(NHQ):
                    w1ts = []
                    for d in range(ND):
                        t = wp1.tile([P, 512], BF16, tag="w1",
                                     name=f"{tagp}w1_{hq}_{d}")
                        nc.sync.dma_start(t[:], w1_ap[hq, d])
                        w1ts.append(t)
                        if hq == 0 and x_dma is not None:
                            x_dma(d)
                    if hq == 0:
                        nc.sync.dma_start(b1_tile[:], b1_ap_of[tagp][:])
                    for hh in range(4):
                        h = hq * 4 + hh
                        ht = hp.tile([P, C], BF16, tag="h", name=f"{tagp}h{h}")
                        psums = [pp.tile([P, n], F32, tag="ps", name=f"psA{ti}")
                                 for ti, (_, n) in enumerate(chs)]
                        if hq == 0:
                            for ti, (o, n) in enumerate(chs):
                                for d in range(ND):
                                    nc.tensor.matmul(
                                        psums[ti][:, :n],
                                        w1ts[d][:, hh * P:(hh + 1) * P],
                                        x_tiles[d][ti][:, :n],
                                        start=(d == 0),
                                        stop=(d == ND - 1),
                                    )
                        else:
                            for d in range(ND):
                                for ti, (o, n) in enumerate(chs):
                                    nc.tensor.matmul(
                                        psums[ti][:, :n],
                                        w1ts[d][:, hh * P:(hh + 1) * P],
                                        x_tiles[d][ti][:, :n],
                                        start=(d == 0),
                                        stop=(d == ND - 1),
                                    )
                        for ti, (o, n) in enumerate(chs):
                            nc.scalar.activation(
                                ht[:, o:o + n],
                                psums[ti][:, :n],
                                mybir.ActivationFunctionType.Gelu,
                                bias=b1_tile[:, h:h + 1],
                            )
                        hts.append(ht)

                # phase 2: yT[dt, tok] = (w2[:,dt].T @ hT + b2[dt]) * scale
                if pre2 is not None:
                    pre2()
                for dt in range(ND):
                    w2ts = []
                    for j in range(NHQ):
                        t = wp2.tile([P, 4, P], BF16, tag="w2",
                                     name=f"{tagp}w2_{dt}_{j}")
                        nc.sync.dma_start(t[:], w2_ap[dt, j])
                        w2ts.append(t)
                    psums = [pp.tile([P, n], F32, tag="ps", name=f"psB{ti}")
                             for ti, (_, n) in enumerate(chs)]
                    for h in range(NH):
                        j, a = divmod(h, 4)
                        for ti, (o, n) in enumerate(chs):
                            nc.tensor.matmul(
                                psums[ti][:, :n],
                                w2ts[j][:, a, :],
                                hts[h][:, o:o + n],
                                start=(h == 0),
                                stop=(h == NH - 1),
                            )
                    for ti, (o, n) in enumerate(chs):
                        pieces = [(0, n)]
                        for po, pn in pieces:
                            ot = op.tile([P, NCHUNK], F32, tag="o", name=f"o{ti}")
                            if sc_tile is not None:
                                nc.vector.scalar_tensor_tensor(
                                    ot[:, :pn],
                                    psums[ti][:, po:po + pn],
                                    b2_tile[:, dt:dt + 1],
                                    sc_tile[:, o + po:o + po + pn],
                                    mybir.AluOpType.add,
                                    mybir.AluOpType.mult,
                                )
                            else:
                                nc.vector.tensor_scalar_add(
                                    ot[:, :pn], psums[ti][:, po:po + pn],
                                    b2_tile[:, dt:dt + 1]
                                )
                            nc.sync.dma_start(
                                y_ap[dt * P:(dt + 1) * P, o + po:o + po + pn],
                                ot[:, :pn])

            b1_ap_of = {"e": b1c, "s": sb1c}

            def pre_expert_phase2():
                # loads needed by the expert epilogue and the upcoming
                # shared phases; issued here so they trail the phase-1 w1
                # stream on the queue instead of competing at t=0
                nc.sync.dma_start(b2t[:], b2c[:])
                nc.sync.dma_start(sct[:], scale[:])
                for d in range(ND):
                    nc.sync.dma_start(sxt[d][0][:], sx[d * P:(d + 1) * P, :])

            def pre_shared_phase2():
                nc.sync.dma_start(sb2t[:], sb2c[:])

            def xg_dma(d):
                # chunk 0 interleaves with the w1 stream (it gates the first
                # matmuls); remaining chunks queue right behind
                nc.sync.dma_start(xgt[d][0][:, :ech[0][1]],
                                  xg[d * P:(d + 1) * P, 0:ech[0][1]])
                if d == ND - 1:
                    for ti in range(1, len(ech)):
                        o, n = ech[ti]
                        for dd in range(ND):
                            nc.sync.dma_start(
                                xgt[dd][ti][:, :n],
                                xg[dd * P:(dd + 1) * P, o:o + n])

            ffn("e", w1, w2, b1t, b2t, xgt, ech, ye, sct,
                pre2=pre_expert_phase2, x_dma=xg_dma)
            ffn("s", sw1, sw2, sb1t, sb2t, sxt, sch, ys, None,
                pre2=pre_shared_phase2)

    nc.compile()
    return nc


def _get_nc(C, S):
    key = (C, S)
    if key not in _cache:
        _cache[key] = build(C, S)
    return _cache[key]


def _pack_w1(w):
    # [D, H] -> [hq, d, 128, 512]
    return np.ascontiguousarray(
        np.asarray(w).reshape(D // P, P, H // 512, 512).transpose(2, 0, 1, 3)
    ).astype(ml_dtypes.bfloat16)


def _pack_w2(w):
    # [H, D] -> [dt, j, 128, 4, 128]
    return np.ascontiguousarray(
        np.asarray(w).reshape(H // 512, 4, P, D // P, P).transpose(3, 0, 2, 1, 4)
    ).astype(ml_dtypes.bfloat16)


def prepare(x, gate_w, gate_b, route_bias, shared_w1, shared_b1, shared_w2,
            shared_b2, exp_w1, exp_b1, exp_w2, exp_b2):
    """Host routing + sharding. Returns (nc, in_maps, combine_fn)."""
    B, SEQ, _ = x.shape
    T = B * SEQ
    S = T // NCORES
    xf = np.ascontiguousarray(x.reshape(T, D)).astype(np.float32)

    # --- gate / routing (this IS the data-dependent shard map) ---
    logits = xf @ np.asarray(gate_w, np.float32) + np.asarray(gate_b, np.float32) \
        + np.asarray(route_bias, np.float32)
    m = logits.max(axis=-1, keepdims=True)
    e = np.exp(logits - m)
    probs = e / e.sum(axis=-1, keepdims=True)
    i1 = probs.argmax(axis=-1)
    p1 = probs[np.arange(T), i1]
    probs2 = probs.copy()
    probs2[np.arange(T), i1] = -np.inf
    i2 = probs2.argmax(axis=-1)
    p2 = probs[np.arange(T), i2]
    den = p1 + p2
    p1n = p1 / den
    p2n = p2 / den

    idx = []
    pv = []
    for ex in range(E):
        sel1 = np.nonzero(i1 == ex)[0]
        sel2 = np.nonzero(i2 == ex)[0]
        idx.append(np.concatenate([sel1, sel2]))
        pv.append(np.concatenate([p1n[sel1], p2n[sel2]]).astype(np.float32))
    counts = np.array([len(ix) for ix in idx])
    # exact capacity rounded to 16 tokens — matmul moving dim and DMA handle
    # arbitrary sizes; only the token-chunking below cares
    C = max(16, int(np.ceil(counts.max() / 16)) * 16)

    xf_bf = xf.astype(ml_dtypes.bfloat16)
    sw1_p = _pack_w1(shared_w1)
    sw2_p = _pack_w2(shared_w2)
    sb1c = np.ascontiguousarray(np.asarray(shared_b1, np.float32).reshape(H // P, P).T)
    sb2c = np.ascontiguousarray(np.asarray(shared_b2, np.float32).reshape(D // P, P).T)

    in_maps = []
    for c in range(NCORES):
        n = counts[c]
        xg = np.zeros((D, C), ml_dtypes.bfloat16)
        xg[:, :n] = xf_bf[idx[c]].T
        sc = np.zeros((P, C), np.float32)
        sc[:, :n] = pv[c][None, :]
        in_maps.append({
            "xg": xg,
            "sx": np.ascontiguousarray(xf_bf[c * S:(c + 1) * S].T),
            "w1": _pack_w1(exp_w1[c]),
            "w2": _pack_w2(exp_w2[c]),
            "sw1": sw1_p,
            "sw2": sw2_p,
            "b1c": np.ascontiguousarray(
                np.asarray(exp_b1[c], np.float32).reshape(H // P, P).T),
            "b2c": np.ascontiguousarray(
                np.asarray(exp_b2[c], np.float32).reshape(D // P, P).T),
            "sb1c": sb1c,
            "sb2c": sb2c,
            "scale": sc,
        })

    nc = _get_nc(C, S)

    def combine(results):
        out = np.zeros((T, D), np.float32)
        for c in range(NCORES):
            out[c * S:(c + 1) * S] = results[c]["ys"].T
        for ex in range(E):
            n = counts[ex]
            out[idx[ex]] += results[ex]["ye"][:, :n].T
        return out.reshape(B, SEQ, D)

    return nc, in_maps, combine


def kernel(**inputs):
    nc, in_maps, combine = prepare(**inputs)
    res = run_bass_kernel_spmd(nc, in_maps, core_ids=list(range(NCORES)))
    return combine(res.results)
